# revision 1
# baseline (speedup 1.0000x reference)
"""Trainium2 Bass kernel for per-sample dynamic (CDNA) depthwise 5x5 conv.

Computation (per sample b):
  k = relu(emb_flat @ W.T + b - 1e-5) + 1e-5        [225] -> [9, 25]
  k = k / k.sum(-1, keepdims=True)                  normalized 5x5 kernels
  out[k,c,h,w] = sum_{i,j} k[k,5i+j] * pad(rgb)[c,h+i,w+j]   [9,3,256,256]

Sharding: data-parallel over batch, 4 samples per core on 8 cores.

Conv-as-matmul mapping ("banded weights"):
  For an output row-tile of HH=14 rows, build lhsT_j [18, 126] with
  lhsT_j[rr, k*14+hh] = kn[k, 5*(rr-hh)+j] (banded in rr-hh). Then
    psum[(k,hh), (c,w)] += sum_rr lhsT_j[rr, (k,hh)] * padded[c, h0+rr, w+j]
  accumulated over j=0..4 gives the full 5x5 conv for 126 output rows at
  once; the rhs is the *same* staged SBUF tile read at free-offset j, so no
  patch replication is needed. The normalization (1/sum) is folded into the
  PSUM evacuation as a per-partition tensor_scalar multiply.

  The banded matrices are built on-device from the FC output with a
  stride-trick DMA: in linear DRAM the diagonal band becomes a constant
  stride (135 elements per hh step), expressible as a plain 3-dim DMA AP.
"""

import sys
import numpy as np

try:
    import concourse  # noqa: F401
except ImportError:
    sys.path.insert(0, "/opt/trn_rl_repo")

KER = 5
NK = 9
SHIFT = 1e-5
B, C, H, W_IMG = 32, 3, 256, 256
PAD = KER // 2
HP = H + 2 * PAD  # 260
NCORES = 8
BL = B // NCORES  # 4 batches per core
FCIN = 8192
FCOUT = NK * KER * KER  # 225
HH = 14           # output rows per conv tile
M_FULL = NK * HH  # 126
KR_FULL = HH + KER - 1  # 18
NTILES = (H + HH - 1) // HH  # 19 (18 full + one 4-row tile)
H_LAST = H - (NTILES - 1) * HH  # 4

USE_F32R = True  # float32r: single-pass fp32 matmul (4x faster than fp32)

_CACHE = {}


def _build_nc(rep=1):
    import concourse.bass as bass
    import concourse.bacc as bacc
    import concourse.mybir as mybir
    from concourse import tile
    from contextlib import ExitStack

    f32 = mybir.dt.float32
    dt_mm = mybir.dt.float32r if USE_F32R else mybir.dt.float32

    def mm_cast(ap):
        return ap.bitcast(dt_mm) if USE_F32R else ap

    nc = bacc.Bacc("TRN2", target_bir_lowering=False, debug=False)

    embt = nc.dram_tensor("embt", [FCIN, BL], f32, kind="ExternalInput").ap()
    wt = nc.dram_tensor("wt", [FCIN, FCOUT], f32, kind="ExternalInput").ap()
    biasm = nc.dram_tensor("biasm", [FCOUT, 1], f32, kind="ExternalInput").ap()
    bones = nc.dram_tensor("bones", [FCOUT, NK], f32, kind="ExternalInput").ap()
    rgbp = nc.dram_tensor("rgbp", [BL, C, HP, HP], dt_mm, kind="ExternalInput").ap()
    out = nc.dram_tensor(
        "out", [BL, NK, C, H, W_IMG], f32, kind="ExternalOutput"
    ).ap()

    # DRAM scratch
    zrd = nc.dram_tensor("zrd", [BL, M_FULL], f32)  # zrd[b, k*14+hh] = 1/Z[k,b]
    knflat = nc.dram_tensor("knflat", [FCOUT, BL], f32)  # fc-major
    banded = [
        nc.dram_tensor(f"banded{b}", [KER, KR_FULL, M_FULL], dt_mm) for b in range(BL)
    ]

    # FC output M split: 225 = 125 (k=0..4) + 100 (k=5..8)
    M0, M1 = 125, 100
    NCHUNK = FCIN // 128  # 64

    with tile.TileContext(nc) as tc, ExitStack() as ctx:
        persist = ctx.enter_context(tc.tile_pool(name="persist", bufs=1))
        conv_in = ctx.enter_context(tc.tile_pool(name="conv_in", bufs=3))
        conv_out = ctx.enter_context(tc.tile_pool(name="conv_out", bufs=3))
        setup = ctx.enter_context(tc.tile_pool(name="setup", bufs=1))

        # ---------------- FC + normalization + banded build ----------------
        wt_sb = setup.tile([128, NCHUNK * FCOUT], f32, tag="wt")
        nc.sync.dma_start(
            wt_sb[:].rearrange("p (c n) -> p c n", c=NCHUNK),
            wt.rearrange("(c p) n -> p c n", p=128),
        )
        embt_sb = setup.tile([128, NCHUNK * BL], f32, tag="embt")
        nc.sync.dma_start(
            embt_sb[:].rearrange("p (c b) -> p c b", c=NCHUNK),
            embt.rearrange("(c p) b -> p c b", p=128),
        )
        biasm_sb0 = setup.tile([M0, 1], f32, tag="biasm0")
        nc.sync.dma_start(biasm_sb0[:], biasm[0:M0])
        biasm_sb1 = setup.tile([M1, 1], f32, tag="biasm1")
        nc.sync.dma_start(biasm_sb1[:], biasm[M0:FCOUT])
        bones_sb0 = setup.tile([M0, NK], f32, tag="bones0")
        nc.sync.dma_start(bones_sb0[:], bones[0:M0])
        bones_sb1 = setup.tile([M1, NK], f32, tag="bones1")
        nc.sync.dma_start(bones_sb1[:], bones[M0:FCOUT])

        psum_fc = ctx.enter_context(
            tc.tile_pool(name="psum_fc", bufs=1, space="PSUM"))


        if True:
            knp0 = psum_fc.tile([M0, BL], f32, tag="knp0")
            knp1 = psum_fc.tile([M1, BL], f32, tag="knp1")
            for ci in range(NCHUNK):
                rhs = embt_sb[:, ci * BL:(ci + 1) * BL]
                nc.tensor.matmul(
                    knp0[:],
                    lhsT=wt_sb[:, ci * FCOUT: ci * FCOUT + M0],
                    rhs=rhs,
                    start=(ci == 0),
                    stop=(ci == NCHUNK - 1),
                )
                nc.tensor.matmul(
                    knp1[:],
                    lhsT=wt_sb[:, ci * FCOUT + M0:(ci + 1) * FCOUT],
                    rhs=rhs,
                    start=(ci == 0),
                    stop=(ci == NCHUNK - 1),
                )

            # knr = relu(fc + bias - shift) + shift
            knr0 = setup.tile([M0, BL], f32, tag="knr0")
            nc.scalar.activation(
                knr0[:], knp0[:], mybir.ActivationFunctionType.Relu,
                bias=biasm_sb0[:],
            )
            nc.vector.tensor_scalar_add(knr0[:], knr0[:], SHIFT)
            knr1 = setup.tile([M1, BL], f32, tag="knr1")
            nc.scalar.activation(
                knr1[:], knp1[:], mybir.ActivationFunctionType.Relu,
                bias=biasm_sb1[:],
            )
            nc.vector.tensor_scalar_add(knr1[:], knr1[:], SHIFT)

            # Z[b, k] = sum_p knr[25k+p, b]
            zps = psum_fc.tile([BL, NK], f32, tag="zps")
            nc.tensor.matmul(zps[:], lhsT=knr0[:], rhs=bones_sb0[:],
                             start=True, stop=False)
            nc.tensor.matmul(zps[:], lhsT=knr1[:], rhs=bones_sb1[:],
                             start=False, stop=True)
            zr = setup.tile([BL, NK], f32, tag="zr")
            nc.vector.reciprocal(zr[:], zps[:])

        # replicate recip along hh (m = hh*9+k order) and bounce through
        # DRAM to build the per-partition normalization vectors rv_b [126, 1]
        zr_rep = setup.tile([BL, HH * NK], f32, tag="zr_rep")
        nc.vector.tensor_copy(
            zr_rep[:].rearrange("b (hh k) -> b hh k", hh=HH),
            zr[:].unsqueeze(1).broadcast_to([BL, HH, NK]),
        )
        nc.sync.dma_start(zrd.ap(), zr_rep[:])
        rv = []
        rv_last = []
        for b in range(BL):
            rv_b = persist.tile([M_FULL, 1], f32, tag=f"rv{b}")
            nc.sync.dma_start(rv_b[:], zrd.ap()[b].unsqueeze(1))
            rv.append(rv_b)
            rv_lb = persist.tile([NK * H_LAST, 1], f32, tag=f"rvl{b}")
            nc.sync.dma_start(
                rv_lb[:], zrd.ap()[b, 0:NK * H_LAST].unsqueeze(1))
            rv_last.append(rv_lb)

        # knr -> knflat[fc_idx, b] in DRAM (plain layout)
        nc.sync.dma_start(knflat.ap()[0:M0], knr0[:])
        nc.sync.dma_start(knflat.ap()[M0:FCOUT], knr1[:])
        # kn_kpre partition p'' = j*5+d, free = k*BL+b: value knflat[25k+5d+j, b]
        kn_kpre = setup.tile([KER * KER, NK * BL], f32, tag="kn_kpre")
        for j in range(KER):
            nc.sync.dma_start(
                kn_kpre[j * KER:(j + 1) * KER].rearrange(
                    "d (k b) -> d k b", k=NK),
                bass.AP(knflat, j * BL,
                        [[KER * BL, KER], [KER * KER * BL, NK], [1, BL]]),
            )
        # kn_k[p', b*126 + hh*9 + k] (hh-replicated, k contiguous)
        kn_k = setup.tile([KER * KER, BL * HH * NK], dt_mm, tag="kn_k")
        nc.vector.tensor_copy(
            kn_k[:].rearrange("p (b hh k) -> p b hh k", b=BL, hh=HH),
            kn_kpre[:].rearrange("p (k b) -> p b k", k=NK)
            .unsqueeze(2).broadcast_to([KER * KER, BL, HH, NK]),
        )

        # zero-init banded matrices, then scatter the band entries
        ztile = setup.tile([KR_FULL, KER * M_FULL], dt_mm, tag="ztile")
        zsrc = setup.tile([KR_FULL, KER * M_FULL], f32, tag="zsrc")
        nc.vector.memset(zsrc[:], 0.0)
        nc.vector.tensor_copy(ztile[:], zsrc[:])
        for b in range(BL):
            nc.sync.dma_start(
                banded[b].ap().rearrange("j r m -> r j m"),
                ztile[:].rearrange("r (j m) -> r j m", j=KER),
            )
            for j in range(KER):
                # dst linear addr = j_off + d*126 + hh*135 + k
                dst = bass.AP(
                    banded[b],
                    j * KR_FULL * M_FULL,
                    [[M_FULL, KER], [M_FULL + NK, HH], [1, NK]],
                )
                src = kn_k[j * KER:(j + 1) * KER,
                           b * HH * NK:(b + 1) * HH * NK].rearrange(
                    "d (hh k) -> d hh k", hh=HH)
                nc.sync.dma_start(dst, src)

        # load banded -> lhsT tiles [18, 5*126]
        lhsT = []
        for b in range(BL):
            lt = persist.tile([KR_FULL, KER * M_FULL], dt_mm, tag=f"lhsT{b}")
            nc.sync.dma_start(
                lt[:].rearrange("r (j m) -> r j m", j=KER),
                banded[b].ap().rearrange("j r m -> r j m"),
            )
            lhsT.append(lt)

        # ---------------- conv main loop ----------------
        with tc.tile_pool(name="psum_conv", bufs=2, space="PSUM") as psum_conv:
          from contextlib import nullcontext
          with (tc.For_i(0, rep, 1) if rep > 1 else nullcontext()):
            for b in range(BL):
                for t in range(NTILES):
                    h0 = t * HH
                    hh = HH if t < NTILES - 1 else H_LAST
                    kr = hh + KER - 1
                    m = NK * hh

                    stage = conv_in.tile([kr, C * HP], dt_mm, tag="stage")
                    nc.sync.dma_start(
                        stage[:].rearrange("r (c w) -> r c w", c=C),
                        rgbp[b, :, h0:h0 + kr, :].rearrange("c h w -> h c w"),
                    )
                    st_v = stage[:].rearrange("r (c w) -> r c w", c=C)

                    psA = psum_conv.tile([m, 2 * W_IMG], f32, tag="psA")
                    psB = psum_conv.tile([m, W_IMG], f32, tag="psB")
                    for j in range(KER):
                        lt_j = lhsT[b][0:kr, j * M_FULL: j * M_FULL + m]
                        rhsA = st_v[:, 0:2, j:j + W_IMG]
                        rhsB = st_v[:, 2, j:j + W_IMG]
                        nc.tensor.matmul(
                            psA[:], lhsT=lt_j, rhs=rhsA,
                            start=(j == 0), stop=(j == KER - 1),
                        )
                        nc.tensor.matmul(
                            psB[:], lhsT=lt_j, rhs=rhsB,
                            start=(j == 0), stop=(j == KER - 1),
                        )

                    rv_ap = rv[b][:] if hh == HH else rv_last[b][:]
                    osb = conv_out.tile([m, C * W_IMG], f32, tag="osb")
                    nc.vector.tensor_scalar(
                        osb[:, 0:2 * W_IMG], psA[:], rv_ap, None,
                        op0=mybir.AluOpType.mult,
                    )
                    nc.vector.tensor_scalar(
                        osb[:, 2 * W_IMG:C * W_IMG], psB[:], rv_ap, None,
                        op0=mybir.AluOpType.mult,
                    )
                    for c in range(C):
                        nc.sync.dma_start(
                            out[b, :, c, h0:h0 + hh, :].rearrange(
                                "k hh w -> hh k w"),
                            osb[:, c * W_IMG:(c + 1) * W_IMG],
                        )
    nc.compile()
    return nc


def _host_prep(emb, rgb, W, b):
    emb_t = np.ascontiguousarray(emb.reshape(B, FCIN).T)  # [8192, 32]
    wt = np.ascontiguousarray(W.T)  # [8192, 225]
    biasm = (b.astype(np.float32) - SHIFT).reshape(FCOUT, 1).copy()
    bones = np.zeros((FCOUT, NK), dtype=np.float32)
    for k in range(NK):
        bones[k * KER * KER:(k + 1) * KER * KER, k] = 1.0
    rgbp = np.pad(rgb, ((0, 0), (0, 0), (PAD, PAD), (PAD, PAD)))
    in_maps = []
    for core in range(NCORES):
        sl = slice(core * BL, (core + 1) * BL)
        in_maps.append({
            "embt": np.ascontiguousarray(emb_t[:, sl]),
            "wt": wt,
            "biasm": biasm,
            "bones": bones,
            "rgbp": np.ascontiguousarray(rgbp[sl]),
        })
    return in_maps


def get_nc(rep=1):
    key = f"nc{rep}"
    if key not in _CACHE:
        _CACHE[key] = _build_nc(rep)
    return _CACHE[key]


def kernel(emb, rgb, W, b):
    from concourse.bass_utils import run_bass_kernel_spmd

    assert emb.shape == (B, 128, 8, 8) and rgb.shape == (B, C, H, W_IMG)
    nc = get_nc()
    in_maps = _host_prep(
        np.asarray(emb, dtype=np.float32),
        np.asarray(rgb, dtype=np.float32),
        np.asarray(W, dtype=np.float32),
        np.asarray(b, dtype=np.float32),
    )
    res = run_bass_kernel_spmd(nc, in_maps, list(range(NCORES)))
    return np.concatenate([r["out"] for r in res.results], axis=0)



# revision 6
# speedup vs baseline: 3.0516x; 3.0516x over previous
"""Trainium2 Bass kernel for per-sample dynamic (CDNA) depthwise 5x5 conv.

Computation (per sample b):
  k = relu(emb_flat @ W.T + b - 1e-5) + 1e-5        [225] -> [9, 25]
  k = k / k.sum(-1, keepdims=True)                  normalized 5x5 kernels
  out[k,c,h,w] = sum_{i,j} k[k,5i+j] * pad(rgb)[c,h+i,w+j]   [9,3,256,256]

Sharding: data-parallel over batch, 4 samples per core on 8 cores.

Conv-as-matmul mapping, single-stream variant: all 25 taps live in the
contraction dim.  For an output row-tile of HH=14 rows the lhsT is a
banded [90, 128] matrix with partition p = r*5 + j (r = input row within
the 18-row strip, j = horizontal tap) and
  lhsT[r*5+j, hh*9+k] = kn[k, 5*(r-hh)+j] / Z[k]   for 0 <= r-hh <= 4.
The rhs [90, N] holds the input strip replicated 5x with horizontal
shifts: rhs[r*5+j, (c,w)] = padded[c, h0+r, w+j].  One matmul per
(sample, psum-bank-chunk) computes 126 output rows (9 kernels x 14 image
rows) in a single stream -- 5x fewer PE cycles than accumulating the 5
horizontal taps.  The replicated rhs is pre-materialized host-side so
each tile needs exactly one big contiguous HBM load.

Everything runs in bf16 (inputs, weights, output) with f32 PSUM
accumulation; the kernel normalization 1/Z is folded into the banded
weights so PSUM evacuation is a pure copy/cast, split across the DVE,
Activation and GpSimd engines.  Output rows are written h-major so each
(tile, sample-pair) evacuation is one strided DMA; the host transposes
[B,H,K,C,W] -> [B,K,C,H,W] at the end.
"""

import sys
import numpy as np

try:
    import concourse  # noqa: F401
except ImportError:
    sys.path.insert(0, "/opt/trn_rl_repo")

import ml_dtypes

BF16 = ml_dtypes.bfloat16

KER = 5
NK = 9
SHIFT = 1e-5
B, C, H, W_IMG = 32, 3, 256, 256
PAD = KER // 2
HPH = H + 2 * PAD           # 260 padded rows
ROWW = W_IMG + 2 * PAD      # 260 useful row width
WPAD = W_IMG + 2 * PAD + 4  # 264 host pad width (shift overflow room)
NCORES = 8
BL = B // NCORES            # 4 samples per core
FCIN = 8192
FCOUT = NK * KER * KER      # 225
HH = 14                     # output rows per conv tile
M_REAL = NK * HH            # 126
MPAD = 128                  # padded lhsT free size (FWL wants 128)
KR = (HH + KER - 1) * KER   # 90 contraction partitions (r*5+j)
NT = 18 + 1                 # 18 full tiles + one overlapping tail tile
H0S = [14 * t for t in range(18)] + [H - HH]  # last tile at 242
TAIL_HH0 = 10               # tail tile only writes hh >= 10 (h 252..255)
NCHUNK = FCIN // 128        # 64

CW = C * ROWW               # 780 free elems per (sample, strip-row)
OUT_HSTRIDE = NK * C * W_IMG    # 6912 elems per output row h
OUT_BSTRIDE = H * OUT_HSTRIDE   # 1769472 elems per sample

_CACHE = {}


def _build_nc():
    import concourse.bass as bass
    import concourse.bacc as bacc
    import concourse.mybir as mybir
    from concourse import tile
    from contextlib import ExitStack

    f32 = mybir.dt.float32
    bf16 = mybir.dt.bfloat16
    AF = mybir.ActivationFunctionType
    ALU = mybir.AluOpType

    nc = bacc.Bacc("TRN2", target_bir_lowering=False, debug=False)

    # per-core external inputs
    rgbrep = nc.dram_tensor("rgbrep", [NT, KR, BL * CW], bf16,
                            kind="ExternalInput")
    wt = nc.dram_tensor("wt", [FCIN, FCOUT], bf16, kind="ExternalInput")
    embt = nc.dram_tensor("embt", [FCIN, BL], bf16, kind="ExternalInput")
    biasbc = nc.dram_tensor("biasbc", [BL, FCOUT], f32, kind="ExternalInput")
    out2 = nc.dram_tensor("out2", [BL * OUT_BSTRIDE], bf16,
                          kind="ExternalOutput")

    # DRAM scratch
    knflat = nc.dram_tensor("knflat", [FCOUT, BL], f32)
    banded = nc.dram_tensor("banded", [BL, KR, MPAD], bf16)

    with tile.TileContext(nc) as tc, ExitStack() as ctx:
        setup = ctx.enter_context(tc.tile_pool(name="setup", bufs=1))
        persist = ctx.enter_context(tc.tile_pool(name="persist", bufs=1))
        rep_pool = ctx.enter_context(tc.tile_pool(name="rep", bufs=3))
        osb_pool = ctx.enter_context(tc.tile_pool(name="osb", bufs=3))

        # ---------------- FC (b-major: M=4, N=225) ----------------
        wt_sb = setup.tile([128, NCHUNK * FCOUT], bf16, tag="wt")
        nc.sync.dma_start(
            wt_sb[:].rearrange("p (c n) -> p c n", c=NCHUNK),
            wt.ap().rearrange("(c p) n -> p c n", p=128),
        )
        embt_sb = setup.tile([128, NCHUNK * BL], bf16, tag="embt")
        nc.sync.dma_start(
            embt_sb[:].rearrange("p (c b) -> p c b", c=NCHUNK),
            embt.ap().rearrange("(c p) b -> p c b", p=128),
        )
        bias_sb = setup.tile([BL, FCOUT], f32, tag="bias")
        nc.sync.dma_start(bias_sb[:], biasbc.ap())

        with tc.tile_pool(name="psum_fc", bufs=1, space="PSUM") as psum_fc:
            kfc = psum_fc.tile([BL, FCOUT], f32, tag="kfc")
            for ci in range(NCHUNK):
                nc.tensor.matmul(
                    kfc[:],
                    lhsT=embt_sb[:, ci * BL:(ci + 1) * BL],
                    rhs=wt_sb[:, ci * FCOUT:(ci + 1) * FCOUT],
                    start=(ci == 0),
                    stop=(ci == NCHUNK - 1),
                )

            # knr = relu(kfc + (bias - shift)) + shift, then fold in 1/Z
            knr = setup.tile([BL, FCOUT], f32, tag="knr")
            nc.vector.tensor_tensor(knr[:], kfc[:], bias_sb[:], op=ALU.add)
        nc.scalar.activation(knr[:], knr[:], AF.Relu)
        nc.vector.tensor_scalar_add(knr[:], knr[:], SHIFT)
        zs = setup.tile([BL, NK], f32, tag="zs")
        nc.vector.reduce_sum(
            zs[:], knr[:].rearrange("b (k p) -> b k p", k=NK),
            axis=mybir.AxisListType.X,
        )
        zr = setup.tile([BL, NK], f32, tag="zr")
        nc.vector.reciprocal(zr[:], zs[:])
        knd_sb = setup.tile([BL, FCOUT], f32, tag="knd")
        nc.vector.tensor_tensor(
            knd_sb[:].rearrange("b (k p) -> b k p", k=NK),
            knr[:].rearrange("b (k p) -> b k p", k=NK),
            zr[:].unsqueeze(2).broadcast_to([BL, NK, KER * KER]),
            op=ALU.mult,
        )

        # knflat[fc, b] via transposing store (tiny, 900 descriptors)
        nc.sync.dma_start(
            bass.AP(knflat, 0, [[1, BL], [BL, FCOUT]]), knd_sb[:])

        # kn_kpre[q=(j*5+d), (k, b)] = knflat[25k+5d+j, b]
        kn_kpre = setup.tile([KER * KER, NK * BL], f32, tag="kn_kpre")
        for j in range(KER):
            nc.sync.dma_start(
                kn_kpre[j * KER:(j + 1) * KER].rearrange(
                    "d (k b) -> d k b", k=NK),
                bass.AP(knflat, j * BL,
                        [[KER * BL, KER], [KER * KER * BL, NK], [1, BL]]),
            )
        # kn_k[q, (b, hh, k)] bf16 (hh-replicated)
        kn_k = setup.tile([KER * KER, BL * HH * NK], bf16, tag="kn_k")
        nc.vector.tensor_copy(
            kn_k[:].rearrange("q (b hh k) -> q b hh k", b=BL, hh=HH),
            kn_kpre[:].rearrange("q (k b) -> q b k", k=NK)
            .unsqueeze(2).broadcast_to([KER * KER, BL, HH, NK]),
        )

        # zero-init banded, then scatter the band entries
        ztile = setup.tile([KR, MPAD * BL // 1], bf16, tag="ztile")
        nc.vector.memset(ztile[:, 0:MPAD * BL], 0.0)
        nc.sync.dma_start(
            banded.ap().rearrange("b p m -> p b m"),
            ztile[:, 0:MPAD * BL].rearrange("p (b m) -> p b m", b=BL),
        )
        for b in range(BL):
            for j in range(KER):
                # banded addr (within sample b): p*MPAD + m,
                # p = (hh+d)*5 + j, m = hh*9 + k
                dst = bass.AP(
                    banded,
                    b * KR * MPAD + j * MPAD,
                    [[KER * MPAD, KER],            # d
                     [KER * MPAD + NK, HH],        # hh
                     [1, NK]],                     # k
                )
                src = kn_k[j * KER:(j + 1) * KER,
                           b * HH * NK:(b + 1) * HH * NK].rearrange(
                    "d (hh k) -> d hh k", hh=HH)
                nc.gpsimd.dma_start(dst, src)

        lhsT = persist.tile([KR, BL * MPAD], bf16, tag="lhsT")
        nc.sync.dma_start(
            lhsT[:].rearrange("p (b m) -> p b m", b=BL),
            banded.ap().rearrange("b p m -> p b m"),
        )

        # ---------------- conv main loop ----------------
        def evac_dve(dst, src):
            nc.vector.tensor_copy(dst, src)

        def evac_act(dst, src):
            nc.scalar.activation(dst, src, AF.Copy)

        # GPSIMD cannot read PSUM; split evacuation DVE:Act = 3:2
        evac_engines = [evac_dve, evac_act, evac_dve, evac_act, evac_dve]
        evac_i = 0
        with tc.tile_pool(name="psum_conv", bufs=2, space="PSUM") as psc:
            for t in range(NT):
                rep = rep_pool.tile([KR, BL * CW], bf16, tag="rep")
                nc.scalar.dma_start(rep[:], rgbrep.ap()[t])
                rv = rep[:].rearrange("p (b c w) -> p b c w", b=BL, c=C)
                for sp in range(2):
                    osb = osb_pool.tile([MPAD, 2 * C * W_IMG], bf16,
                                        tag="osb")
                    for bl in range(2):
                        b = 2 * sp + bl
                        ps = psc.tile([MPAD, C * W_IMG], f32, tag=f"ps{bl}")
                        lt = lhsT[:, b * MPAD:(b + 1) * MPAD]
                        nc.tensor.matmul(
                            ps[:, 0:2 * W_IMG], lhsT=lt,
                            rhs=rv[:, b, 0:2, 0:W_IMG],
                            start=True, stop=True,
                        )
                        nc.tensor.matmul(
                            ps[:, 2 * W_IMG:C * W_IMG], lhsT=lt,
                            rhs=rv[:, b, 2, 0:W_IMG],
                            start=True, stop=True,
                        )
                        eng = evac_engines[evac_i % 5]
                        evac_i += 1
                        eng(osb[:, bl * C * W_IMG:(bl + 1) * C * W_IMG],
                            ps[:])
                    if t < NT - 1:
                        nc.sync.dma_start(
                            bass.AP(out2,
                                    2 * sp * OUT_BSTRIDE
                                    + H0S[t] * OUT_HSTRIDE,
                                    [[C * W_IMG, M_REAL],
                                     [OUT_BSTRIDE, 2],
                                     [1, C * W_IMG]]),
                            osb[0:M_REAL, :],
                        )
                    else:
                        nc.sync.dma_start(
                            bass.AP(out2,
                                    2 * sp * OUT_BSTRIDE
                                    + (H0S[t] + TAIL_HH0) * OUT_HSTRIDE,
                                    [[C * W_IMG, M_REAL - TAIL_HH0 * NK],
                                     [OUT_BSTRIDE, 2],
                                     [1, C * W_IMG]]),
                            osb[TAIL_HH0 * NK:M_REAL, :],
                        )
    nc.compile()
    return nc


def _host_prep(emb, rgb, W, b):
    emb_t = emb.reshape(B, FCIN).T.astype(BF16)          # [8192, 32]
    wt = W.T.astype(BF16)                                # [8192, 225]
    biasbc = np.broadcast_to((b.astype(np.float32) - SHIFT)[None, :],
                             (BL, FCOUT)).copy()

    # replicated+shifted conv rhs: rep[t, r*5+j, b, c*260+w] =
    #   padded[b, c, h0[t]+r, w+j]
    padded = np.pad(rgb, ((0, 0), (0, 0), (PAD, PAD),
                          (PAD, PAD + 4))).astype(BF16)  # [32,3,260,264]
    sw = np.lib.stride_tricks.sliding_window_view(
        padded, ROWW, axis=3)                            # [32,3,260,5,260]
    idx = np.asarray(H0S)[:, None] + np.arange(HH + KER - 1)[None, :]
    g = sw[:, :, idx]                                    # [32,3,19,18,5,260]
    repf = np.ascontiguousarray(
        g.transpose(2, 3, 4, 0, 1, 5)).reshape(NT, KR, B, CW)

    in_maps = []
    for core in range(NCORES):
        sl = slice(core * BL, (core + 1) * BL)
        in_maps.append({
            "rgbrep": np.ascontiguousarray(repf[:, :, sl]).reshape(
                NT, KR, BL * CW),
            "wt": wt,
            "embt": np.ascontiguousarray(emb_t[:, sl]),
            "biasbc": biasbc,
        })
    return in_maps


def get_nc():
    if "nc" not in _CACHE:
        _CACHE["nc"] = _build_nc()
    return _CACHE["nc"]


def kernel(emb, rgb, W, b):
    from concourse.bass_utils import run_bass_kernel_spmd

    emb = np.asarray(emb, dtype=np.float32)
    rgb = np.asarray(rgb, dtype=np.float32)
    W = np.asarray(W, dtype=np.float32)
    b = np.asarray(b, dtype=np.float32)
    assert emb.shape == (B, 128, 8, 8) and rgb.shape == (B, C, H, W_IMG)

    nc = get_nc()
    in_maps = _host_prep(emb, rgb, W, b)
    res = run_bass_kernel_spmd(nc, in_maps, list(range(NCORES)))
    outs = []
    for r in res.results:
        o = np.asarray(r["out2"]).reshape(BL, H, NK, C, W_IMG)
        outs.append(o)
    full = np.concatenate(outs, axis=0)                  # [32,H,K,C,W] bf16
    return full.transpose(0, 2, 3, 1, 4).astype(np.float32)


# revision 13
# speedup vs baseline: 3.5684x; 1.1693x over previous
"""Trainium2 Bass kernel for per-sample dynamic (CDNA) depthwise 5x5 conv.

Computation (per sample b):
  k = relu(emb_flat @ W.T + b - 1e-5) + 1e-5        [225] -> [9, 25]
  k = k / k.sum(-1, keepdims=True)                  normalized 5x5 kernels
  out[k,c,h,w] = sum_{i,j} k[k,5i+j] * pad(rgb)[c,h+i,w+j]   [9,3,256,256]

Sharding: data-parallel over batch, 4 samples per core on 8 cores.

Conv-as-matmul mapping, single-stream variant: all 25 taps live in the
contraction dim.  For an output row-tile of HH=14 rows the lhsT is a
banded [90, 128] matrix with partition p = r*5 + j (r = input row within
the 18-row strip, j = horizontal tap) and
  lhsT[r*5+j, hh*9+k] = kn[k, 5*(r-hh)+j] / Z[k]   for 0 <= r-hh <= 4.
The rhs [90, N] holds the input strip replicated 5x with horizontal
shifts: rhs[r*5+j, (c,w)] = padded[c, h0+r, w+j].  One matmul per
(sample, psum-bank-chunk) computes 126 output rows (9 kernels x 14 image
rows) in a single stream -- 5x fewer PE cycles than accumulating the 5
horizontal taps.  The replicated rhs is pre-materialized host-side so
each tile needs exactly one big contiguous HBM load.

Everything runs in bf16 (inputs, weights, output) with f32 PSUM
accumulation; the kernel normalization 1/Z is folded into the banded
weights so PSUM evacuation is a pure copy/cast, split across the DVE,
Activation and GpSimd engines.  Output rows are written h-major so each
(tile, sample-pair) evacuation is one strided DMA; the host transposes
[B,H,K,C,W] -> [B,K,C,H,W] at the end.
"""

import sys
import numpy as np

try:
    import concourse  # noqa: F401
except ImportError:
    sys.path.insert(0, "/opt/trn_rl_repo")

import ml_dtypes

BF16 = ml_dtypes.bfloat16

KER = 5
NK = 9
SHIFT = 1e-5
B, C, H, W_IMG = 32, 3, 256, 256
PAD = KER // 2
HPH = H + 2 * PAD           # 260 padded rows
ROWW = W_IMG + 2 * PAD      # 260 useful row width
WPAD = W_IMG + 2 * PAD + 4  # 264 host pad width (shift overflow room)
NCORES = 8
BL = B // NCORES            # 4 samples per core
FCIN = 8192
FCOUT = NK * KER * KER      # 225
HH = 14                     # output rows per conv tile
M_REAL = NK * HH            # 126
MPAD = 128                  # padded lhsT free size (FWL wants 128)
KR = (HH + KER - 1) * KER   # 90 contraction partitions (r*5+j)
NT = 18 + 1                 # 18 full tiles + one overlapping tail tile
H0S = [14 * t for t in range(18)] + [H - HH]  # last tile at 242
TAIL_HH0 = 10               # tail tile only writes hh >= 10 (h 252..255)
NCHUNK = FCIN // 128        # 64

CW = C * ROWW               # 780 free elems per (sample, strip-row)
OUT_HSTRIDE = NK * C * W_IMG    # 6912 elems per output row h
OUT_BSTRIDE = H * OUT_HSTRIDE   # 1769472 elems per sample

_CACHE = {}


def _build_nc():
    import concourse.bass as bass
    import concourse.bacc as bacc
    import concourse.mybir as mybir
    from concourse import tile
    from contextlib import ExitStack

    f32 = mybir.dt.float32
    bf16 = mybir.dt.bfloat16
    AF = mybir.ActivationFunctionType
    ALU = mybir.AluOpType

    nc = bacc.Bacc("TRN2", target_bir_lowering=False, debug=False)

    # per-core external inputs.  wt/embt come pre-swizzled host-side so the
    # SBUF load is one contiguous run per partition (128 descriptors, not
    # 8192): wt2[p, c, n] = W.T[c*128+p, n].
    rgbrep = nc.dram_tensor("rgbrep", [NT, KR, BL * CW], bf16,
                            kind="ExternalInput")
    wt = nc.dram_tensor("wt", [128, NCHUNK * FCOUT], bf16,
                        kind="ExternalInput")
    embt = nc.dram_tensor("embt", [128, NCHUNK * BL], bf16,
                          kind="ExternalInput")
    biasbc = nc.dram_tensor("biasbc", [BL, FCOUT], f32, kind="ExternalInput")
    # raw dump of the per-tile output staging tiles; host reassembles
    out2 = nc.dram_tensor("out2", [NT, M_REAL, 2 * 2 * C * W_IMG], bf16,
                          kind="ExternalOutput")

    # DRAM scratch
    knflat = nc.dram_tensor("knflat", [FCOUT, BL], f32)
    banded = nc.dram_tensor("banded", [BL, KR, MPAD], bf16)

    with tile.TileContext(nc) as tc, ExitStack() as ctx:
        setup = ctx.enter_context(tc.tile_pool(name="setup", bufs=1))
        persist = ctx.enter_context(tc.tile_pool(name="persist", bufs=1))
        rep_pool = ctx.enter_context(tc.tile_pool(name="rep", bufs=3))
        osb_pool = ctx.enter_context(tc.tile_pool(name="osb", bufs=3))

        # ---------------- FC (b-major: M=4, N=225) ----------------
        wt_sb = setup.tile([128, NCHUNK * FCOUT], bf16, tag="wt")
        nc.sync.dma_start(wt_sb[:], wt.ap())
        embt_sb = setup.tile([128, NCHUNK * BL], bf16, tag="embt")
        nc.sync.dma_start(embt_sb[:], embt.ap())
        bias_sb = setup.tile([BL, FCOUT], f32, tag="bias")
        nc.sync.dma_start(bias_sb[:], biasbc.ap())

        with tc.tile_pool(name="psum_fc", bufs=1, space="PSUM") as psum_fc:
            kfc = psum_fc.tile([BL, FCOUT], f32, tag="kfc")
            for ci in range(NCHUNK):
                nc.tensor.matmul(
                    kfc[:],
                    lhsT=embt_sb[:, ci * BL:(ci + 1) * BL],
                    rhs=wt_sb[:, ci * FCOUT:(ci + 1) * FCOUT],
                    start=(ci == 0),
                    stop=(ci == NCHUNK - 1),
                )

            # knr = relu(kfc + (bias - shift)) + shift, then fold in 1/Z
            knr = setup.tile([BL, FCOUT], f32, tag="knr")
            nc.vector.tensor_tensor(knr[:], kfc[:], bias_sb[:], op=ALU.add)
        nc.scalar.activation(knr[:], knr[:], AF.Relu)
        nc.vector.tensor_scalar_add(knr[:], knr[:], SHIFT)
        zs = setup.tile([BL, NK], f32, tag="zs")
        nc.vector.reduce_sum(
            zs[:], knr[:].rearrange("b (k p) -> b k p", k=NK),
            axis=mybir.AxisListType.X,
        )
        zr = setup.tile([BL, NK], f32, tag="zr")
        nc.vector.reciprocal(zr[:], zs[:])
        knd_sb = setup.tile([BL, FCOUT], f32, tag="knd")
        nc.vector.tensor_tensor(
            knd_sb[:].rearrange("b (k p) -> b k p", k=NK),
            knr[:].rearrange("b (k p) -> b k p", k=NK),
            zr[:].unsqueeze(2).broadcast_to([BL, NK, KER * KER]),
            op=ALU.mult,
        )

        # knflat[fc, b] via transposing store (tiny, 900 descriptors)
        nc.sync.dma_start(
            bass.AP(knflat, 0, [[1, BL], [BL, FCOUT]]), knd_sb[:])

        # kn_kpre[q=(j*5+d), (k, b)] = knflat[25k+5d+j, b]
        kn_kpre = setup.tile([KER * KER, NK * BL], f32, tag="kn_kpre")
        for j in range(KER):
            nc.sync.dma_start(
                kn_kpre[j * KER:(j + 1) * KER].rearrange(
                    "d (k b) -> d k b", k=NK),
                bass.AP(knflat, j * BL,
                        [[KER * BL, KER], [KER * KER * BL, NK], [1, BL]]),
            )
        # kn_k[q, (b, hh, k)] bf16 (hh-replicated)
        kn_k = setup.tile([KER * KER, BL * HH * NK], bf16, tag="kn_k")
        nc.vector.tensor_copy(
            kn_k[:].rearrange("q (b hh k) -> q b hh k", b=BL, hh=HH),
            kn_kpre[:].rearrange("q (k b) -> q b k", k=NK)
            .unsqueeze(2).broadcast_to([KER * KER, BL, HH, NK]),
        )

        # zero-init banded, then scatter the band entries
        ztile = setup.tile([KR, MPAD * BL // 1], bf16, tag="ztile")
        nc.vector.memset(ztile[:, 0:MPAD * BL], 0.0)
        nc.sync.dma_start(
            banded.ap().rearrange("b p m -> p b m"),
            ztile[:, 0:MPAD * BL].rearrange("p (b m) -> p b m", b=BL),
        )
        scat_engines = [nc.sync, nc.scalar]
        for b in range(BL):
            for j in range(KER):
                # banded addr (within sample b): p*MPAD + m,
                # p = (hh+d)*5 + j, m = hh*9 + k
                dst = bass.AP(
                    banded,
                    b * KR * MPAD + j * MPAD,
                    [[KER * MPAD, KER],            # d
                     [KER * MPAD + NK, HH],        # hh
                     [1, NK]],                     # k
                )
                src = kn_k[j * KER:(j + 1) * KER,
                           b * HH * NK:(b + 1) * HH * NK].rearrange(
                    "d (hh k) -> d hh k", hh=HH)
                scat_engines[(b * KER + j) % 2].dma_start(dst, src)

        lhsT = persist.tile([KR, BL * MPAD], bf16, tag="lhsT")
        nc.sync.dma_start(
            lhsT[:].rearrange("p (b m) -> p b m", b=BL),
            banded.ap().rearrange("b p m -> p b m"),
        )

        # ---------------- conv main loop ----------------
        def evac_dve(dst, src):
            nc.vector.tensor_copy(dst, src)

        def evac_act(dst, src):
            nc.scalar.activation(dst, src, AF.Copy)

        # GPSIMD cannot read PSUM; split evacuation DVE:Act = 3:2
        evac_engines = [evac_dve, evac_act, evac_dve, evac_act, evac_dve]
        evac_i = 0
        with tc.tile_pool(name="psum_conv", bufs=2, space="PSUM") as psc:
            for t in range(NT):
                rep = rep_pool.tile([KR, BL * CW], bf16, tag="rep")
                nc.scalar.dma_start(rep[:], rgbrep.ap()[t])
                rv = rep[:].rearrange("p (b c w) -> p b c w", b=BL, c=C)
                osb = osb_pool.tile([MPAD, BL * C * W_IMG], bf16, tag="osb")
                for sp in range(2):
                    for bl in range(2):
                        b = 2 * sp + bl
                        ps = psc.tile([MPAD, C * W_IMG], f32, tag=f"ps{bl}")
                        lt = lhsT[:, b * MPAD:(b + 1) * MPAD]
                        nc.tensor.matmul(
                            ps[:, 0:2 * W_IMG], lhsT=lt,
                            rhs=rv[:, b, 0:2, 0:W_IMG],
                            start=True, stop=True,
                        )
                        nc.tensor.matmul(
                            ps[:, 2 * W_IMG:C * W_IMG], lhsT=lt,
                            rhs=rv[:, b, 2, 0:W_IMG],
                            start=True, stop=True,
                        )
                        eng = evac_engines[evac_i % 5]
                        evac_i += 1
                        eng(osb[:, b * C * W_IMG:(b + 1) * C * W_IMG],
                            ps[:])
                # one contiguous dump per tile (126 x 12KB descriptors)
                nc.sync.dma_start(out2.ap()[t], osb[0:M_REAL, :])
    nc.compile()
    return nc


def _host_prep(emb, rgb, W, b):
    # wt2[p, c, n] = W.T[c*128+p, n]; embt2[p, c, b] = emb_flat[c*128+p, b]
    # -> the SBUF load is one contiguous 28.8KB/0.5KB run per partition.
    wt2 = np.ascontiguousarray(
        W.T.astype(BF16).reshape(NCHUNK, 128, FCOUT).transpose(1, 0, 2)
    ).reshape(128, NCHUNK * FCOUT)
    emb_t = emb.reshape(B, FCIN).T.astype(BF16)          # [8192, 32]
    biasbc = np.broadcast_to((b.astype(np.float32) - SHIFT)[None, :],
                             (BL, FCOUT)).copy()

    # replicated+shifted conv rhs: rep[t, r*5+j, b, c*260+w] =
    #   padded[b, c, h0[t]+r, w+j]
    padded = np.pad(rgb, ((0, 0), (0, 0), (PAD, PAD),
                          (PAD, PAD + 4))).astype(BF16)  # [32,3,260,264]
    sw = np.lib.stride_tricks.sliding_window_view(
        padded, ROWW, axis=3)                            # [32,3,260,5,260]
    idx = np.asarray(H0S)[:, None] + np.arange(HH + KER - 1)[None, :]
    g = sw[:, :, idx]                                    # [32,3,19,18,5,260]
    repf = np.ascontiguousarray(
        g.transpose(2, 3, 4, 0, 1, 5)).reshape(NT, KR, B, CW)

    in_maps = []
    for core in range(NCORES):
        sl = slice(core * BL, (core + 1) * BL)
        embt2 = np.ascontiguousarray(
            emb_t[:, sl].reshape(NCHUNK, 128, BL).transpose(1, 0, 2)
        ).reshape(128, NCHUNK * BL)
        in_maps.append({
            "rgbrep": np.ascontiguousarray(repf[:, :, sl]).reshape(
                NT, KR, BL * CW),
            "wt": wt2,
            "embt": embt2,
            "biasbc": biasbc,
        })
    return in_maps


def _assemble(raw_outs):
    """raw_outs: per-core [NT, M_REAL, BL*C*W] bf16 dumps -> [B,K,C,H,W] f32."""
    full = np.empty((B, NK, C, H, W_IMG), dtype=np.float32)
    for core, o in enumerate(raw_outs):
        # [t, (hh k), (b c w)] -> [t, hh, k, b, c, w]
        o = np.asarray(o).reshape(NT, HH, NK, BL, C, W_IMG)
        sl = slice(core * BL, (core + 1) * BL)
        v = o.transpose(0, 3, 2, 4, 1, 5)        # [t, b, k, c, hh, w]
        for t in range(NT - 1):
            full[sl, :, :, H0S[t]:H0S[t] + HH, :] = v[t]
        full[sl, :, :, H - (HH - TAIL_HH0):, :] = v[NT - 1][:, :, :,
                                                           TAIL_HH0:, :]
    return full


def get_nc():
    if "nc" not in _CACHE:
        _CACHE["nc"] = _build_nc()
    return _CACHE["nc"]


def kernel(emb, rgb, W, b):
    from concourse.bass_utils import run_bass_kernel_spmd

    emb = np.asarray(emb, dtype=np.float32)
    rgb = np.asarray(rgb, dtype=np.float32)
    W = np.asarray(W, dtype=np.float32)
    b = np.asarray(b, dtype=np.float32)
    assert emb.shape == (B, 128, 8, 8) and rgb.shape == (B, C, H, W_IMG)

    nc = get_nc()
    in_maps = _host_prep(emb, rgb, W, b)
    res = run_bass_kernel_spmd(nc, in_maps, list(range(NCORES)))
    return _assemble([r["out2"] for r in res.results])


# revision 22
# speedup vs baseline: 4.0514x; 1.1354x over previous
"""Trainium2 Bass kernel for per-sample dynamic (CDNA) depthwise 5x5 conv.

Computation (per sample b):
  k = relu(emb_flat @ W.T + b - 1e-5) + 1e-5        [225] -> [9, 25]
  k = k / k.sum(-1, keepdims=True)                  normalized 5x5 kernels
  out[k,c,h,w] = sum_{i,j} k[k,5i+j] * pad(rgb)[c,h+i,w+j]   [9,3,256,256]

Sharding: data-parallel over batch, 4 samples per core on 8 cores.

Conv-as-matmul mapping, single-stream variant: all 25 taps live in the
contraction dim.  For an output row-tile of HH=14 rows the lhsT is a
banded [90, 128] matrix with partition p = r*5 + j (r = input row within
the 18-row strip, j = horizontal tap) and
  lhsT[r*5+j, hh*9+k] = kn[k, 5*(r-hh)+j] / Z[k]   for 0 <= r-hh <= 4.
The rhs [90, N] holds the input strip replicated 5x with horizontal
shifts: rhs[r*5+j, (c,w)] = padded[c, h0+r, w+j].  One matmul per
(sample, psum-bank-chunk) computes 126 output rows (9 kernels x 14 image
rows) in a single stream -- 5x fewer PE cycles than accumulating the 5
horizontal taps.  The replicated rhs is pre-materialized host-side so
each tile needs exactly one big contiguous HBM load.

Everything runs in bf16 (inputs, weights, output) with f32 PSUM
accumulation; the kernel normalization 1/Z is folded into the banded
weights so PSUM evacuation is a pure copy/cast, split across the DVE,
Activation and GpSimd engines.  Output rows are written h-major so each
(tile, sample-pair) evacuation is one strided DMA; the host transposes
[B,H,K,C,W] -> [B,K,C,H,W] at the end.
"""

import sys
import numpy as np

try:
    import concourse  # noqa: F401
except ImportError:
    sys.path.insert(0, "/opt/trn_rl_repo")

import ml_dtypes

BF16 = ml_dtypes.bfloat16

KER = 5
NK = 9
SHIFT = 1e-5
B, C, H, W_IMG = 32, 3, 256, 256
PAD = KER // 2
HPH = H + 2 * PAD           # 260 padded rows
ROWW = W_IMG + 2 * PAD      # 260 useful row width
WPAD = W_IMG + 2 * PAD + 4  # 264 host pad width (shift overflow room)
NCORES = 8
BL = B // NCORES            # 4 samples per core
FCIN = 8192
FCOUT = NK * KER * KER      # 225
HH = 14                     # output rows per conv tile
M_REAL = NK * HH            # 126
MPAD = 128                  # padded lhsT free size (FWL wants 128)
KR = (HH + KER - 1) * KER   # 90 contraction partitions (r*5+j)
NT = 18 + 1                 # 18 full tiles + one overlapping tail tile
H0S = [14 * t for t in range(18)] + [H - HH]  # last tile at 242
TAIL_HH0 = 10               # tail tile only writes hh >= 10 (h 252..255)
NCHUNK = FCIN // 128        # 64

CW = C * ROWW               # 780 free elems per (sample, strip-row)
OUT_HSTRIDE = NK * C * W_IMG    # 6912 elems per output row h
OUT_BSTRIDE = H * OUT_HSTRIDE   # 1769472 elems per sample

_CACHE = {}


def _build_nc():
    import concourse.bass as bass
    import concourse.bacc as bacc
    import concourse.mybir as mybir
    from concourse import tile
    from contextlib import ExitStack

    f32 = mybir.dt.float32
    bf16 = mybir.dt.bfloat16
    AF = mybir.ActivationFunctionType
    ALU = mybir.AluOpType

    nc = bacc.Bacc("TRN2", target_bir_lowering=False, debug=False)

    # per-core external inputs.  wt/embt come pre-swizzled host-side so the
    # SBUF load is one contiguous run per partition (128 descriptors, not
    # 8192): wt2[p, c, n] = W.T[c*128+p, n].
    rgbrep = nc.dram_tensor("rgbrep", [NT, KR, BL * CW], bf16,
                            kind="ExternalInput")
    wt = nc.dram_tensor("wt", [128, NCHUNK * FCOUT], bf16,
                        kind="ExternalInput")
    embt = nc.dram_tensor("embt", [128, NCHUNK * BL], bf16,
                          kind="ExternalInput")
    biasbc = nc.dram_tensor("biasbc", [BL, FCOUT], f32, kind="ExternalInput")
    # raw dump of the per-tile output staging tiles; host reassembles
    out2 = nc.dram_tensor("out2", [NT, M_REAL, 2 * 2 * C * W_IMG], bf16,
                          kind="ExternalOutput")

    maskb = nc.dram_tensor("maskb", [KR, MPAD], bf16, kind="ExternalInput")
    # DRAM scratch: compact permuted kernels, padded so the banded gather's
    # out-of-band reads stay in-bounds (masked to zero afterwards)
    KOFF = 640
    knpd = nc.dram_tensor("knpd", [3072], bf16)  # = KOFF + 900 + tail pad

    with tile.TileContext(nc) as tc, ExitStack() as ctx:
        setup = ctx.enter_context(tc.tile_pool(name="setup", bufs=1))
        persist = ctx.enter_context(tc.tile_pool(name="persist", bufs=1))
        rep_pool = ctx.enter_context(tc.tile_pool(name="rep", bufs=3))
        osb_pool = ctx.enter_context(tc.tile_pool(name="osb", bufs=3))

        # ---------------- FC (b-major: M=4, N=225) ----------------
        # wt loads in 4 chunks so the FC matmuls can start after the first
        # quarter instead of waiting for the whole 3.7MB.
        NWC = 4
        CPW = NCHUNK // NWC  # 16 fc-chunks per wt tile
        wt_a = []
        for wi in range(NWC):
            wtile = setup.tile([128, CPW * FCOUT], bf16, tag=f"wt{wi}")
            nc.sync.dma_start(
                wtile[:], bass.AP(wt, wi * CPW * FCOUT,
                                  [[NCHUNK * FCOUT, 128], [1, CPW * FCOUT]]))
            wt_a.append(wtile)
        embt_sb = setup.tile([128, NCHUNK * BL], bf16, tag="embt")
        nc.sync.dma_start(embt_sb[:], embt.ap())
        bias_sb = setup.tile([BL, FCOUT], f32, tag="bias")
        nc.sync.dma_start(bias_sb[:], biasbc.ap())

        with tc.tile_pool(name="psum_fc", bufs=1, space="PSUM") as psum_fc:
            kfc = psum_fc.tile([BL, FCOUT], f32, tag="kfc")
            for ci in range(NCHUNK):
                nc.tensor.matmul(
                    kfc[:],
                    lhsT=embt_sb[:, ci * BL:(ci + 1) * BL],
                    rhs=wt_a[ci // CPW][:, (ci % CPW) * FCOUT:
                                        (ci % CPW + 1) * FCOUT],
                    start=(ci == 0),
                    stop=(ci == NCHUNK - 1),
                )

            # knr = relu(kfc + (bias - shift)) + shift, then fold in 1/Z
            knr = setup.tile([BL, FCOUT], f32, tag="knr")
            nc.vector.tensor_tensor(knr[:], kfc[:], bias_sb[:], op=ALU.add)
        nc.scalar.activation(knr[:], knr[:], AF.Relu)
        nc.vector.tensor_scalar_add(knr[:], knr[:], SHIFT)
        zs = setup.tile([BL, NK], f32, tag="zs")
        nc.vector.reduce_sum(
            zs[:], knr[:].rearrange("b (k p) -> b k p", k=NK),
            axis=mybir.AxisListType.X,
        )
        zr = setup.tile([BL, NK], f32, tag="zr")
        nc.vector.reciprocal(zr[:], zs[:])
        knd_sb = setup.tile([BL, FCOUT], f32, tag="knd")
        nc.vector.tensor_tensor(
            knd_sb[:].rearrange("b (k p) -> b k p", k=NK),
            knr[:].rearrange("b (k p) -> b k p", k=NK),
            zr[:].unsqueeze(2).broadcast_to([BL, NK, KER * KER]),
            op=ALU.mult,
        )

        # permute fc -> (d, j, k) within the free dim + cast to bf16
        knp = setup.tile([BL, FCOUT], bf16, tag="knp")
        nc.vector.tensor_copy(
            knp[:].rearrange("b (d j k) -> b d j k", d=KER, j=KER),
            knd_sb[:].rearrange("b (k d j) -> b d j k", k=NK, d=KER),
        )

        # zero-fill the knpd pad regions (off the critical chain), then
        # store the compact kernels in the middle
        zt2 = setup.tile([128, 24], bf16, tag="zt2")
        nc.vector.memset(zt2[:], 0.0)
        nc.scalar.dma_start(
            bass.AP(knpd, 0, [[24, 128], [1, 24]]), zt2[:])
        nc.sync.dma_start(
            bass.AP(knpd, KOFF, [[FCOUT, BL], [1, FCOUT]]), knp[:])

        # banded gather: lhsT_raw[p, b, hh*9+k] = knpd[OFF + 225b + 9p
        #   - 45hh + 9k]  (garbage out-of-band; masked below)
        lhsT_raw = persist.tile([KR, BL * MPAD], bf16, tag="lhsT_raw")
        nc.vector.memset(lhsT_raw[:], 0.0)
        gat_engines = [nc.sync, nc.scalar]
        for b in range(BL):
            gat_engines[b % 2].dma_start(
                lhsT_raw[:, b * MPAD:b * MPAD + NK * HH].rearrange(
                    "p (hh k) -> p hh k", hh=HH),
                bass.AP(knpd, KOFF + b * FCOUT,
                        [[NK, KR], [-KER * NK, HH], [1, NK]]),
            )
        mask_sb = setup.tile([KR, MPAD], bf16, tag="mask")
        nc.sync.dma_start(mask_sb[:], maskb.ap())
        lhsT = persist.tile([KR, BL * MPAD], bf16, tag="lhsT")
        nc.vector.tensor_tensor(
            lhsT[:].rearrange("p (b m) -> p b m", b=BL),
            lhsT_raw[:].rearrange("p (b m) -> p b m", b=BL),
            mask_sb[:].unsqueeze(1).broadcast_to([KR, BL, MPAD]),
            op=ALU.mult,
        )

        # ---------------- conv main loop ----------------
        def evac_dve(dst, src):
            nc.vector.tensor_copy(dst, src)

        def evac_act(dst, src):
            nc.scalar.activation(dst, src, AF.Copy)

        # GPSIMD cannot read PSUM; split evacuation DVE:Act = 3:2
        evac_engines = [evac_dve, evac_act, evac_dve, evac_act, evac_dve]
        evac_i = 0
        with tc.tile_pool(name="psum_conv", bufs=2, space="PSUM") as psc:
            for t in range(NT):
                rep = rep_pool.tile([KR, BL * CW], bf16, tag="rep")
                nc.scalar.dma_start(rep[:], rgbrep.ap()[t])
                rv = rep[:].rearrange("p (b c w) -> p b c w", b=BL, c=C)
                osb = osb_pool.tile([MPAD, BL * C * W_IMG], bf16, tag="osb")
                for sp in range(2):
                    for bl in range(2):
                        b = 2 * sp + bl
                        ps = psc.tile([MPAD, C * W_IMG], f32, tag=f"ps{bl}")
                        lt = lhsT[:, b * MPAD:(b + 1) * MPAD]
                        nc.tensor.matmul(
                            ps[:, 0:2 * W_IMG], lhsT=lt,
                            rhs=rv[:, b, 0:2, 0:W_IMG],
                            start=True, stop=True,
                        )
                        nc.tensor.matmul(
                            ps[:, 2 * W_IMG:C * W_IMG], lhsT=lt,
                            rhs=rv[:, b, 2, 0:W_IMG],
                            start=True, stop=True,
                        )
                        eng = evac_engines[evac_i % 5]
                        evac_i += 1
                        eng(osb[:, b * C * W_IMG:(b + 1) * C * W_IMG],
                            ps[:])
                # one contiguous dump per tile (126 x 12KB descriptors)
                nc.sync.dma_start(out2.ap()[t], osb[0:M_REAL, :])
    nc.compile()
    return nc


def _host_prep(emb, rgb, W, b):
    # wt2[p, c, n] = W.T[c*128+p, n]; embt2[p, c, b] = emb_flat[c*128+p, b]
    # -> the SBUF load is one contiguous 28.8KB/0.5KB run per partition.
    wt2 = np.ascontiguousarray(
        W.T.astype(BF16).reshape(NCHUNK, 128, FCOUT).transpose(1, 0, 2)
    ).reshape(128, NCHUNK * FCOUT)
    # band mask: maskb[p, hh*9+k] = 1 iff 0 <= p//5 - hh <= 4
    maskb = np.zeros((KR, MPAD), dtype=BF16)
    for p in range(KR):
        for hh in range(HH):
            if 0 <= p // KER - hh <= KER - 1:
                maskb[p, hh * NK:(hh + 1) * NK] = 1
    emb_t = emb.reshape(B, FCIN).T.astype(BF16)          # [8192, 32]
    biasbc = np.broadcast_to((b.astype(np.float32) - SHIFT)[None, :],
                             (BL, FCOUT)).copy()

    # replicated+shifted conv rhs: rep[t, r*5+j, b, c*260+w] =
    #   padded[b, c, h0[t]+r, w+j]
    padded = np.pad(rgb, ((0, 0), (0, 0), (PAD, PAD),
                          (PAD, PAD + 4))).astype(BF16)  # [32,3,260,264]
    sw = np.lib.stride_tricks.sliding_window_view(
        padded, ROWW, axis=3)                            # [32,3,260,5,260]
    idx = np.asarray(H0S)[:, None] + np.arange(HH + KER - 1)[None, :]
    g = sw[:, :, idx]                                    # [32,3,19,18,5,260]
    repf = np.ascontiguousarray(
        g.transpose(2, 3, 4, 0, 1, 5)).reshape(NT, KR, B, CW)

    in_maps = []
    for core in range(NCORES):
        sl = slice(core * BL, (core + 1) * BL)
        embt2 = np.ascontiguousarray(
            emb_t[:, sl].reshape(NCHUNK, 128, BL).transpose(1, 0, 2)
        ).reshape(128, NCHUNK * BL)
        in_maps.append({
            "rgbrep": np.ascontiguousarray(repf[:, :, sl]).reshape(
                NT, KR, BL * CW),
            "wt": wt2,
            "embt": embt2,
            "biasbc": biasbc,
            "maskb": maskb,
        })
    return in_maps


def _assemble(raw_outs):
    """raw_outs: per-core [NT, M_REAL, BL*C*W] bf16 dumps -> [B,K,C,H,W] f32."""
    full = np.empty((B, NK, C, H, W_IMG), dtype=np.float32)
    for core, o in enumerate(raw_outs):
        # [t, (hh k), (b c w)] -> [t, hh, k, b, c, w]
        o = np.asarray(o).reshape(NT, HH, NK, BL, C, W_IMG)
        sl = slice(core * BL, (core + 1) * BL)
        v = o.transpose(0, 3, 2, 4, 1, 5)        # [t, b, k, c, hh, w]
        for t in range(NT - 1):
            full[sl, :, :, H0S[t]:H0S[t] + HH, :] = v[t]
        full[sl, :, :, H - (HH - TAIL_HH0):, :] = v[NT - 1][:, :, :,
                                                           TAIL_HH0:, :]
    return full


def get_nc():
    if "nc" not in _CACHE:
        _CACHE["nc"] = _build_nc()
    return _CACHE["nc"]


def kernel(emb, rgb, W, b):
    from concourse.bass_utils import run_bass_kernel_spmd

    emb = np.asarray(emb, dtype=np.float32)
    rgb = np.asarray(rgb, dtype=np.float32)
    W = np.asarray(W, dtype=np.float32)
    b = np.asarray(b, dtype=np.float32)
    assert emb.shape == (B, 128, 8, 8) and rgb.shape == (B, C, H, W_IMG)

    nc = get_nc()
    in_maps = _host_prep(emb, rgb, W, b)
    res = run_bass_kernel_spmd(nc, in_maps, list(range(NCORES)))
    return _assemble([r["out2"] for r in res.results])


# revision 27
# speedup vs baseline: 4.4804x; 1.1059x over previous
"""Trainium2 Bass kernel for per-sample dynamic (CDNA) depthwise 5x5 conv.

Computation (per sample b):
  k = relu(emb_flat @ W.T + b - 1e-5) + 1e-5        [225] -> [9, 25]
  k = k / k.sum(-1, keepdims=True)                  normalized 5x5 kernels
  out[k,c,h,w] = sum_{i,j} k[k,5i+j] * pad(rgb)[c,h+i,w+j]   [9,3,256,256]

Sharding: data-parallel over batch, 4 samples per core on 8 cores.

Conv-as-matmul mapping, single-stream variant: all 25 taps live in the
contraction dim.  For an output row-tile of HH=14 rows the lhsT is a
banded [90, 128] matrix with partition p = r*5 + j (r = input row within
the 18-row strip, j = horizontal tap) and
  lhsT[r*5+j, hh*9+k] = kn[k, 5*(r-hh)+j] / Z[k]   for 0 <= r-hh <= 4.
The rhs [90, N] holds the input strip replicated 5x with horizontal
shifts: rhs[r*5+j, (c,w)] = padded[c, h0+r, w+j].  One matmul per
(sample, psum-bank-chunk) computes 126 output rows (9 kernels x 14 image
rows) in a single stream -- 5x fewer PE cycles than accumulating the 5
horizontal taps.  The replicated rhs is pre-materialized host-side so
each tile needs exactly one big contiguous HBM load.

Everything runs in bf16 (inputs, weights, output) with f32 PSUM
accumulation; the kernel normalization 1/Z is folded into the banded
weights so PSUM evacuation is a pure copy/cast, split across the DVE,
Activation and GpSimd engines.  Output rows are written h-major so each
(tile, sample-pair) evacuation is one strided DMA; the host transposes
[B,H,K,C,W] -> [B,K,C,H,W] at the end.
"""

import sys
import numpy as np

try:
    import concourse  # noqa: F401
except ImportError:
    sys.path.insert(0, "/opt/trn_rl_repo")

import ml_dtypes

BF16 = ml_dtypes.bfloat16

KER = 5
NK = 9
SHIFT = 1e-5
B, C, H, W_IMG = 32, 3, 256, 256
PAD = KER // 2
HPH = H + 2 * PAD           # 260 padded rows
ROWW = W_IMG + 2 * PAD      # 260 useful row width
WPAD = W_IMG + 2 * PAD + 4  # 264 host pad width (shift overflow room)
NCORES = 8
BL = B // NCORES            # 4 samples per core
FCIN = 8192
FCOUT = NK * KER * KER      # 225
HH = 14                     # output rows per conv tile
M_REAL = NK * HH            # 126
MPAD = 128                  # padded lhsT free size (FWL wants 128)
KR = (HH + KER - 1) * KER   # 90 contraction partitions (r*5+j)
NT = 18 + 1                 # 18 full tiles + one overlapping tail tile
H0S = [14 * t for t in range(18)] + [H - HH]  # last tile at 242
TAIL_HH0 = 10               # tail tile only writes hh >= 10 (h 252..255)
NCHUNK = FCIN // 128        # 64

CW = C * ROWW               # 780 free elems per (sample, strip-row)
OUT_HSTRIDE = NK * C * W_IMG    # 6912 elems per output row h
OUT_BSTRIDE = H * OUT_HSTRIDE   # 1769472 elems per sample

_CACHE = {}


def _build_nc():
    import concourse.bass as bass
    import concourse.bacc as bacc
    import concourse.mybir as mybir
    from concourse import tile
    from contextlib import ExitStack

    f32 = mybir.dt.float32
    bf16 = mybir.dt.bfloat16
    AF = mybir.ActivationFunctionType
    ALU = mybir.AluOpType

    nc = bacc.Bacc("TRN2", target_bir_lowering=False, debug=False)

    # per-core external inputs.  wt/embt come pre-swizzled host-side so the
    # SBUF load is one contiguous run per partition (128 descriptors, not
    # 8192): wt2[p, c, n] = W.T[c*128+p, n].
    rgbrep = nc.dram_tensor("rgbrep", [NT, KR, BL * CW], bf16,
                            kind="ExternalInput")
    wt = nc.dram_tensor("wt", [128, NCHUNK * FCOUT], bf16,
                        kind="ExternalInput")
    embt = nc.dram_tensor("embt", [128, NCHUNK * BL], bf16,
                          kind="ExternalInput")
    biasbc = nc.dram_tensor("biasbc", [BL, FCOUT], f32, kind="ExternalInput")
    # raw dump of the per-tile output staging tiles; host reassembles
    out2 = nc.dram_tensor("out2", [NT, M_REAL, 2 * 2 * C * W_IMG], bf16,
                          kind="ExternalOutput")

    maskb = nc.dram_tensor("maskb", [KR, MPAD], bf16, kind="ExternalInput")
    # DRAM scratch: compact permuted kernels, padded so the banded gather's
    # out-of-band reads stay in-bounds (masked to zero afterwards)
    KOFF = 640
    knpd = nc.dram_tensor("knpd", [3072], bf16)  # = KOFF + 900 + tail pad

    with tile.TileContext(nc) as tc, ExitStack() as ctx:
        setup = ctx.enter_context(tc.tile_pool(name="setup", bufs=1))
        persist = ctx.enter_context(tc.tile_pool(name="persist", bufs=1))
        rep_pool = ctx.enter_context(tc.tile_pool(name="rep", bufs=3))
        osb_pool = ctx.enter_context(tc.tile_pool(name="osb", bufs=3))

        # ---------------- FC (b-major: M=4, N=225) ----------------
        # small FC inputs first (they gate the first FC matmul), then the wt
        # chunks split across both HWDGE rings.
        embt_sb = setup.tile([128, NCHUNK * BL], bf16, tag="embt")
        nc.sync.dma_start(embt_sb[:], embt.ap())
        bias_sb = setup.tile([BL, FCOUT], f32, tag="bias")
        nc.scalar.dma_start(bias_sb[:], biasbc.ap())
        mask_sb = setup.tile([KR, MPAD], bf16, tag="mask")
        nc.scalar.dma_start(mask_sb[:], maskb.ap())
        NWC = 4
        CPW = NCHUNK // NWC  # 16 fc-chunks per wt tile
        wt_a = []
        for wi in range(NWC):
            wtile = setup.tile([128, CPW * FCOUT], bf16, tag=f"wt{wi}")
            (nc.sync if wi % 2 == 0 else nc.scalar).dma_start(
                wtile[:], bass.AP(wt, wi * CPW * FCOUT,
                                  [[NCHUNK * FCOUT, 128], [1, CPW * FCOUT]]))
            wt_a.append(wtile)

        with tc.tile_pool(name="psum_fc", bufs=1, space="PSUM") as psum_fc:
            kfc = psum_fc.tile([BL, FCOUT], f32, tag="kfc")
            for ci in range(NCHUNK):
                nc.tensor.matmul(
                    kfc[:],
                    lhsT=embt_sb[:, ci * BL:(ci + 1) * BL],
                    rhs=wt_a[ci // CPW][:, (ci % CPW) * FCOUT:
                                        (ci % CPW + 1) * FCOUT],
                    start=(ci == 0),
                    stop=(ci == NCHUNK - 1),
                )

            # knr = relu(kfc + (bias - shift)) + shift, then fold in 1/Z
            knr = setup.tile([BL, FCOUT], f32, tag="knr")
            nc.vector.tensor_tensor(knr[:], kfc[:], bias_sb[:], op=ALU.add)
        nc.scalar.activation(knr[:], knr[:], AF.Relu)
        nc.vector.tensor_scalar_add(knr[:], knr[:], SHIFT)
        zs = setup.tile([BL, NK], f32, tag="zs")
        nc.vector.reduce_sum(
            zs[:], knr[:].rearrange("b (k p) -> b k p", k=NK),
            axis=mybir.AxisListType.X,
        )
        zr = setup.tile([BL, NK], f32, tag="zr")
        nc.vector.reciprocal(zr[:], zs[:])
        knd_sb = setup.tile([BL, FCOUT], f32, tag="knd")
        nc.vector.tensor_tensor(
            knd_sb[:].rearrange("b (k p) -> b k p", k=NK),
            knr[:].rearrange("b (k p) -> b k p", k=NK),
            zr[:].unsqueeze(2).broadcast_to([BL, NK, KER * KER]),
            op=ALU.mult,
        )

        # permute fc -> (d, j, k) within the free dim + cast to bf16
        knp = setup.tile([BL, FCOUT], bf16, tag="knp")
        nc.vector.tensor_copy(
            knp[:].rearrange("b (d j k) -> b d j k", d=KER, j=KER),
            knd_sb[:].rearrange("b (k d j) -> b d j k", k=NK, d=KER),
        )

        # zero-fill the knpd pad regions (off the critical chain), then
        # store the compact kernels in the middle
        zt2 = setup.tile([128, 24], bf16, tag="zt2")
        nc.vector.memset(zt2[:], 0.0)
        nc.scalar.dma_start(
            bass.AP(knpd, 0, [[24, 128], [1, 24]]), zt2[:])
        nc.sync.dma_start(
            bass.AP(knpd, KOFF, [[FCOUT, BL], [1, FCOUT]]), knp[:])

        # banded gather: lhsT_raw[p, b, hh*9+k] = knpd[OFF + 225b + 9p
        #   - 45hh + 9k]  (garbage out-of-band; masked below)
        lhsT_raw = persist.tile([KR, BL * MPAD], bf16, tag="lhsT_raw")
        nc.vector.memset(lhsT_raw[:], 0.0)
        gat_engines = [nc.sync, nc.scalar]
        for b in range(BL):
            gat_engines[b % 2].dma_start(
                lhsT_raw[:, b * MPAD:b * MPAD + NK * HH].rearrange(
                    "p (hh k) -> p hh k", hh=HH),
                bass.AP(knpd, KOFF + b * FCOUT,
                        [[NK, KR], [-KER * NK, HH], [1, NK]]),
            )
        lhsT = persist.tile([KR, BL * MPAD], bf16, tag="lhsT")
        nc.vector.tensor_tensor(
            lhsT[:].rearrange("p (b m) -> p b m", b=BL),
            lhsT_raw[:].rearrange("p (b m) -> p b m", b=BL),
            mask_sb[:].unsqueeze(1).broadcast_to([KR, BL, MPAD]),
            op=ALU.mult,
        )

        # ---------------- conv main loop ----------------
        def evac_dve(dst, src):
            nc.vector.tensor_copy(dst, src)

        def evac_act(dst, src):
            nc.scalar.activation(dst, src, AF.Copy)

        # GPSIMD cannot read PSUM; alternate evacuation DVE/Act
        evac_engines = [evac_dve, evac_act]
        evac_i = 0
        with tc.tile_pool(name="psum_conv", bufs=2, space="PSUM") as psc:
            for t in range(NT):
                rep = rep_pool.tile([KR, BL * CW], bf16, tag="rep")
                nc.gpsimd.dma_start(rep[:], rgbrep.ap()[t])
                rv = rep[:].rearrange("p (b c w) -> p b c w", b=BL, c=C)
                osb = osb_pool.tile([MPAD, BL * C * W_IMG], bf16, tag="osb")
                for sp in range(2):
                    for bl in range(2):
                        b = 2 * sp + bl
                        ps = psc.tile([MPAD, C * W_IMG], f32, tag=f"ps{bl}")
                        lt = lhsT[:, b * MPAD:(b + 1) * MPAD]
                        nc.tensor.matmul(
                            ps[:, 0:2 * W_IMG], lhsT=lt,
                            rhs=rv[:, b, 0:2, 0:W_IMG],
                            start=True, stop=True,
                        )
                        nc.tensor.matmul(
                            ps[:, 2 * W_IMG:C * W_IMG], lhsT=lt,
                            rhs=rv[:, b, 2, 0:W_IMG],
                            start=True, stop=True,
                        )
                        eng = evac_engines[evac_i % 2]
                        evac_i += 1
                        eng(osb[:, b * C * W_IMG:(b + 1) * C * W_IMG],
                            ps[:])
                # one contiguous dump per tile (126 x 12KB descriptors)
                nc.sync.dma_start(out2.ap()[t], osb[0:M_REAL, :])
    nc.compile()
    return nc


def _host_prep(emb, rgb, W, b):
    # wt2[p, c, n] = W.T[c*128+p, n]; embt2[p, c, b] = emb_flat[c*128+p, b]
    # -> the SBUF load is one contiguous 28.8KB/0.5KB run per partition.
    wt2 = np.ascontiguousarray(
        W.T.astype(BF16).reshape(NCHUNK, 128, FCOUT).transpose(1, 0, 2)
    ).reshape(128, NCHUNK * FCOUT)
    # band mask: maskb[p, hh*9+k] = 1 iff 0 <= p//5 - hh <= 4
    maskb = np.zeros((KR, MPAD), dtype=BF16)
    for p in range(KR):
        for hh in range(HH):
            if 0 <= p // KER - hh <= KER - 1:
                maskb[p, hh * NK:(hh + 1) * NK] = 1
    emb_t = emb.reshape(B, FCIN).T.astype(BF16)          # [8192, 32]
    biasbc = np.broadcast_to((b.astype(np.float32) - SHIFT)[None, :],
                             (BL, FCOUT)).copy()

    # replicated+shifted conv rhs: rep[t, r*5+j, b, c*260+w] =
    #   padded[b, c, h0[t]+r, w+j]
    padded = np.pad(rgb, ((0, 0), (0, 0), (PAD, PAD),
                          (PAD, PAD + 4))).astype(BF16)  # [32,3,260,264]
    sw = np.lib.stride_tricks.sliding_window_view(
        padded, ROWW, axis=3)                            # [32,3,260,5,260]
    idx = np.asarray(H0S)[:, None] + np.arange(HH + KER - 1)[None, :]
    g = sw[:, :, idx]                                    # [32,3,19,18,5,260]
    repf = np.ascontiguousarray(
        g.transpose(2, 3, 4, 0, 1, 5)).reshape(NT, KR, B, CW)

    in_maps = []
    for core in range(NCORES):
        sl = slice(core * BL, (core + 1) * BL)
        embt2 = np.ascontiguousarray(
            emb_t[:, sl].reshape(NCHUNK, 128, BL).transpose(1, 0, 2)
        ).reshape(128, NCHUNK * BL)
        in_maps.append({
            "rgbrep": np.ascontiguousarray(repf[:, :, sl]).reshape(
                NT, KR, BL * CW),
            "wt": wt2,
            "embt": embt2,
            "biasbc": biasbc,
            "maskb": maskb,
        })
    return in_maps


def _assemble(raw_outs):
    """raw_outs: per-core [NT, M_REAL, BL*C*W] bf16 dumps -> [B,K,C,H,W] f32."""
    full = np.empty((B, NK, C, H, W_IMG), dtype=np.float32)
    for core, o in enumerate(raw_outs):
        # [t, (hh k), (b c w)] -> [t, hh, k, b, c, w]
        o = np.asarray(o).reshape(NT, HH, NK, BL, C, W_IMG)
        sl = slice(core * BL, (core + 1) * BL)
        v = o.transpose(0, 3, 2, 4, 1, 5)        # [t, b, k, c, hh, w]
        for t in range(NT - 1):
            full[sl, :, :, H0S[t]:H0S[t] + HH, :] = v[t]
        full[sl, :, :, H - (HH - TAIL_HH0):, :] = v[NT - 1][:, :, :,
                                                           TAIL_HH0:, :]
    return full


def get_nc():
    if "nc" not in _CACHE:
        _CACHE["nc"] = _build_nc()
    return _CACHE["nc"]


def kernel(emb, rgb, W, b):
    from concourse.bass_utils import run_bass_kernel_spmd

    emb = np.asarray(emb, dtype=np.float32)
    rgb = np.asarray(rgb, dtype=np.float32)
    W = np.asarray(W, dtype=np.float32)
    b = np.asarray(b, dtype=np.float32)
    assert emb.shape == (B, 128, 8, 8) and rgb.shape == (B, C, H, W_IMG)

    nc = get_nc()
    in_maps = _host_prep(emb, rgb, W, b)
    res = run_bass_kernel_spmd(nc, in_maps, list(range(NCORES)))
    return _assemble([r["out2"] for r in res.results])


# revision 34
# speedup vs baseline: 4.9357x; 1.1016x over previous
"""Trainium2 Bass kernel for per-sample dynamic (CDNA) depthwise 5x5 conv.

Computation (per sample b):
  k = relu(emb_flat @ W.T + b - 1e-5) + 1e-5        [225] -> [9, 25]
  k = k / k.sum(-1, keepdims=True)                  normalized 5x5 kernels
  out[k,c,h,w] = sum_{i,j} k[k,5i+j] * pad(rgb)[c,h+i,w+j]   [9,3,256,256]

Sharding: data-parallel over batch, 4 samples per core on 8 cores.

Conv-as-matmul mapping, single-stream variant: all 25 taps live in the
contraction dim.  For an output row-tile of HH=14 rows the lhsT is a
banded [90, 128] matrix with partition p = r*5 + j (r = input row within
the 18-row strip, j = horizontal tap) and
  lhsT[r*5+j, hh*9+k] = kn[k, 5*(r-hh)+j] / Z[k]   for 0 <= r-hh <= 4.
The rhs [90, N] holds the input strip replicated 5x with horizontal
shifts: rhs[r*5+j, (c,w)] = padded[c, h0+r, w+j].  One matmul per
(sample, psum-bank-chunk) computes 126 output rows (9 kernels x 14 image
rows) in a single stream -- 5x fewer PE cycles than accumulating the 5
horizontal taps.  The replicated rhs is pre-materialized host-side so
each tile needs exactly one big contiguous HBM load.

Everything runs in bf16 (inputs, weights, output) with f32 PSUM
accumulation; the kernel normalization 1/Z is folded into the banded
weights so PSUM evacuation is a pure copy/cast, split across the DVE,
Activation and GpSimd engines.  Output rows are written h-major so each
(tile, sample-pair) evacuation is one strided DMA; the host transposes
[B,H,K,C,W] -> [B,K,C,H,W] at the end.
"""

import sys
import numpy as np

try:
    import concourse  # noqa: F401
except ImportError:
    sys.path.insert(0, "/opt/trn_rl_repo")

import ml_dtypes

BF16 = ml_dtypes.bfloat16

KER = 5
NK = 9
SHIFT = 1e-5
B, C, H, W_IMG = 32, 3, 256, 256
PAD = KER // 2
HPH = H + 2 * PAD           # 260 padded rows
ROWW = W_IMG + 2 * PAD      # 260 useful row width
WPAD = W_IMG + 2 * PAD + 4  # 264 host pad width (shift overflow room)
NCORES = 8
BL = B // NCORES            # 4 samples per core
FCIN = 8192
FCOUT = NK * KER * KER      # 225
HH = 14                     # output rows per conv tile
M_REAL = NK * HH            # 126
MPAD = 128                  # padded lhsT free size (FWL wants 128)
KR = (HH + KER - 1) * KER   # 90 contraction partitions (r*5+j)
NT = 18 + 1                 # 18 full tiles + one overlapping tail tile
H0S = [14 * t for t in range(18)] + [H - HH]  # last tile at 242
TAIL_HH0 = 10               # tail tile only writes hh >= 10 (h 252..255)
NCHUNK = FCIN // 128        # 64

CW = C * ROWW               # 780 free elems per (sample, strip-row)
OUT_HSTRIDE = NK * C * W_IMG    # 6912 elems per output row h
OUT_BSTRIDE = H * OUT_HSTRIDE   # 1769472 elems per sample

_CACHE = {}


def _build_nc():
    import concourse.bass as bass
    import concourse.bacc as bacc
    import concourse.mybir as mybir
    from concourse import tile
    from contextlib import ExitStack

    f32 = mybir.dt.float32
    bf16 = mybir.dt.bfloat16
    AF = mybir.ActivationFunctionType
    ALU = mybir.AluOpType

    nc = bacc.Bacc("TRN2", target_bir_lowering=False, debug=False)

    # per-core external inputs.  wt/embt come pre-swizzled host-side so the
    # SBUF load is one contiguous run per partition (128 descriptors, not
    # 8192): wt2[p, c, n] = W.T[c*128+p, n].
    rgbrep = nc.dram_tensor("rgbrep", [NT, KR, BL * CW], bf16,
                            kind="ExternalInput")
    wt = nc.dram_tensor("wt", [128, NCHUNK * FCOUT], bf16,
                        kind="ExternalInput")
    embt = nc.dram_tensor("embt", [128, NCHUNK * BL], bf16,
                          kind="ExternalInput")
    biasbc = nc.dram_tensor("biasbc", [BL, FCOUT], f32, kind="ExternalInput")
    # raw dump of the per-tile output staging tiles; host reassembles
    out2 = nc.dram_tensor("out2", [NT, M_REAL, 2 * 2 * C * W_IMG], bf16,
                          kind="ExternalOutput")

    maskb = nc.dram_tensor("maskb", [KR, MPAD], bf16, kind="ExternalInput")
    # DRAM scratch: compact permuted kernels, padded so the banded gather's
    # out-of-band reads stay in-bounds (masked to zero afterwards)
    KOFF = 640
    knpd = nc.dram_tensor("knpd", [3072], bf16)  # = KOFF + 900 + tail pad

    with tile.TileContext(nc) as tc, ExitStack() as ctx:
        setup = ctx.enter_context(tc.tile_pool(name="setup", bufs=1))
        persist = ctx.enter_context(tc.tile_pool(name="persist", bufs=1))
        rep_pool = ctx.enter_context(tc.tile_pool(name="rep", bufs=4))
        osb_pool = ctx.enter_context(tc.tile_pool(name="osb", bufs=3))

        # ---------------- FC (b-major: M=4, N=225) ----------------
        # small FC inputs first (they gate the first FC matmul), then the wt
        # chunks split across both HWDGE rings.
        embt_sb = setup.tile([128, NCHUNK * BL], bf16, tag="embt")
        nc.sync.dma_start(embt_sb[:], embt.ap())
        bias_sb = setup.tile([BL, FCOUT], f32, tag="bias")
        nc.scalar.dma_start(bias_sb[:], biasbc.ap())
        mask_sb = setup.tile([KR, MPAD], bf16, tag="mask")
        nc.scalar.dma_start(mask_sb[:], maskb.ap())
        NWC = 6
        CPW = 11  # fc-chunks per wt tile (6*11 > 64; last tile smaller)
        wt_engines = [nc.sync, nc.scalar, nc.gpsimd]
        wt_a = []
        for wi in range(NWC):
            cw = min(CPW, NCHUNK - wi * CPW)
            wtile = setup.tile([128, cw * FCOUT], bf16, tag=f"wt{wi}")
            wt_engines[wi % 3].dma_start(
                wtile[:], bass.AP(wt, wi * CPW * FCOUT,
                                  [[NCHUNK * FCOUT, 128], [1, cw * FCOUT]]))
            wt_a.append(wtile)

        with tc.tile_pool(name="psum_fc", bufs=1, space="PSUM") as psum_fc:
            kfc = psum_fc.tile([BL, FCOUT], f32, tag="kfc")
            for ci in range(NCHUNK):
                nc.tensor.matmul(
                    kfc[:],
                    lhsT=embt_sb[:, ci * BL:(ci + 1) * BL],
                    rhs=wt_a[ci // CPW][:, (ci % CPW) * FCOUT:
                                        (ci % CPW + 1) * FCOUT],
                    start=(ci == 0),
                    stop=(ci == NCHUNK - 1),
                )

            # knr = relu(kfc + (bias - shift)) + shift, then fold in 1/Z
            knr = setup.tile([BL, FCOUT], f32, tag="knr")
            nc.vector.tensor_tensor(knr[:], kfc[:], bias_sb[:], op=ALU.add)
        nc.vector.tensor_scalar(knr[:], knr[:], 0.0, SHIFT,
                                op0=ALU.max, op1=ALU.add)
        zs = setup.tile([BL, NK], f32, tag="zs")
        nc.vector.reduce_sum(
            zs[:], knr[:].rearrange("b (k p) -> b k p", k=NK),
            axis=mybir.AxisListType.X,
        )
        zr = setup.tile([BL, NK], f32, tag="zr")
        nc.vector.reciprocal(zr[:], zs[:])
        knd_sb = setup.tile([BL, FCOUT], f32, tag="knd")
        nc.vector.tensor_tensor(
            knd_sb[:].rearrange("b (k p) -> b k p", k=NK),
            knr[:].rearrange("b (k p) -> b k p", k=NK),
            zr[:].unsqueeze(2).broadcast_to([BL, NK, KER * KER]),
            op=ALU.mult,
        )

        # permute fc -> (d, j, k) within the free dim + cast to bf16
        knp = setup.tile([BL, FCOUT], bf16, tag="knp")
        nc.vector.tensor_copy(
            knp[:].rearrange("b (d j k) -> b d j k", d=KER, j=KER),
            knd_sb[:].rearrange("b (k d j) -> b d j k", k=NK, d=KER),
        )

        # zero-fill the knpd pad regions (off the critical chain), then
        # store the compact kernels in the middle
        zt2 = setup.tile([128, 24], bf16, tag="zt2")
        nc.vector.memset(zt2[:], 0.0)
        nc.scalar.dma_start(
            bass.AP(knpd, 0, [[24, 128], [1, 24]]), zt2[:])
        nc.sync.dma_start(
            bass.AP(knpd, KOFF, [[FCOUT, BL], [1, FCOUT]]), knp[:])

        # banded lhsT via one windowed load + one fused strided mask-multiply.
        # hh runs REVERSED in the output rows (m = (13-hh)*9 + k) so all view
        # strides stay positive:
        #   lhsT[p, b, hh'*9+k] = knpd[KOFF-585 + 9p + 225b + 45hh' + 9k]
        #                       = win[p, 225b + 45hh' + 9k],  masked in-band.
        WINW = 1344
        win = persist.tile([KR, WINW], bf16, tag="win")
        nc.sync.dma_start(
            win[:], bass.AP(knpd, KOFF - 585, [[NK, KR], [1, WINW]]))
        lhsT = persist.tile([KR, BL * MPAD], bf16, tag="lhsT")
        nc.vector.memset(lhsT[:], 0.0)
        nc.vector.tensor_tensor(
            bass.AP(lhsT[:].tensor, 0,
                    [[BL * MPAD, KR], [MPAD, BL], [NK, HH], [1, NK]]),
            bass.AP(win[:].tensor, 0,
                    [[WINW, KR], [FCOUT, BL], [KER * NK, HH], [1, NK]]),
            bass.AP(mask_sb[:].tensor, 0,
                    [[MPAD, KR], [0, BL], [NK, HH], [1, NK]]),
            op=ALU.mult,
        )

        # ---------------- conv main loop ----------------
        def evac_dve(dst, src):
            nc.vector.tensor_copy(dst, src)

        def evac_act(dst, src):
            nc.scalar.activation(dst, src, AF.Copy)

        # GPSIMD cannot read PSUM; alternate evacuation DVE/Act
        evac_engines = [evac_dve, evac_act]
        evac_i = 0
        with tc.tile_pool(name="psum_conv", bufs=2, space="PSUM") as psc:
            for t in range(NT):
                rep = rep_pool.tile([KR, BL * CW], bf16, tag="rep")
                nc.gpsimd.dma_start(rep[:], rgbrep.ap()[t])
                rv = rep[:].rearrange("p (b c w) -> p b c w", b=BL, c=C)
                osb = osb_pool.tile([MPAD, BL * C * W_IMG], bf16, tag="osb")
                for sp in range(2):
                    for bl in range(2):
                        b = 2 * sp + bl
                        ps = psc.tile([MPAD, C * W_IMG], f32, tag=f"ps{bl}")
                        lt = lhsT[:, b * MPAD:(b + 1) * MPAD]
                        nc.tensor.matmul(
                            ps[:, 0:2 * W_IMG], lhsT=lt,
                            rhs=rv[:, b, 0:2, 0:W_IMG],
                            start=True, stop=True,
                        )
                        nc.tensor.matmul(
                            ps[:, 2 * W_IMG:C * W_IMG], lhsT=lt,
                            rhs=rv[:, b, 2, 0:W_IMG],
                            start=True, stop=True,
                        )
                        eng = evac_engines[evac_i % 2]
                        evac_i += 1
                        eng(osb[:, b * C * W_IMG:(b + 1) * C * W_IMG],
                            ps[:])
                # one contiguous dump per tile (126 x 6KB descriptors);
                # the tail tile only has 36 fresh rows (hh'=0..3)
                if t < NT - 1:
                    nc.sync.dma_start(out2.ap()[t], osb[0:M_REAL, :])
                else:
                    nrow = (HH - TAIL_HH0) * NK
                    nc.sync.dma_start(
                        bass.AP(out2, t * M_REAL * BL * C * W_IMG,
                                [[BL * C * W_IMG, nrow], [1, BL * C * W_IMG]]),
                        osb[0:nrow, :])
    nc.compile()
    return nc


def _host_prep(emb, rgb, W, b):
    # wt2[p, c, n] = W.T[c*128+p, n]; embt2[p, c, b] = emb_flat[c*128+p, b]
    # -> the SBUF load is one contiguous 28.8KB/0.5KB run per partition.
    wt2 = np.ascontiguousarray(
        W.T.astype(BF16).reshape(NCHUNK, 128, FCOUT).transpose(1, 0, 2)
    ).reshape(128, NCHUNK * FCOUT)
    # band mask (hh reversed): maskb[p, hh'*9+k] = 1 iff
    # 0 <= p//5 - (13-hh') <= 4
    maskb = np.zeros((KR, MPAD), dtype=BF16)
    for p in range(KR):
        for hp in range(HH):
            if 0 <= p // KER - (HH - 1 - hp) <= KER - 1:
                maskb[p, hp * NK:(hp + 1) * NK] = 1
    emb_t = emb.reshape(B, FCIN).T.astype(BF16)          # [8192, 32]
    biasbc = np.broadcast_to((b.astype(np.float32) - SHIFT)[None, :],
                             (BL, FCOUT)).copy()

    # replicated+shifted conv rhs: rep[t, r*5+j, b, c*260+w] =
    #   padded[b, c, h0[t]+r, w+j]
    padded = np.pad(rgb, ((0, 0), (0, 0), (PAD, PAD),
                          (PAD, PAD + 4))).astype(BF16)  # [32,3,260,264]
    sw = np.lib.stride_tricks.sliding_window_view(
        padded, ROWW, axis=3)                            # [32,3,260,5,260]
    idx = np.asarray(H0S)[:, None] + np.arange(HH + KER - 1)[None, :]
    g = sw[:, :, idx]                                    # [32,3,19,18,5,260]
    repf = np.ascontiguousarray(
        g.transpose(2, 3, 4, 0, 1, 5)).reshape(NT, KR, B, CW)

    in_maps = []
    for core in range(NCORES):
        sl = slice(core * BL, (core + 1) * BL)
        embt2 = np.ascontiguousarray(
            emb_t[:, sl].reshape(NCHUNK, 128, BL).transpose(1, 0, 2)
        ).reshape(128, NCHUNK * BL)
        in_maps.append({
            "rgbrep": np.ascontiguousarray(repf[:, :, sl]).reshape(
                NT, KR, BL * CW),
            "wt": wt2,
            "embt": embt2,
            "biasbc": biasbc,
            "maskb": maskb,
        })
    return in_maps


def _assemble(raw_outs):
    """raw_outs: per-core [NT, M_REAL, BL*C*W] bf16 dumps -> [B,K,C,H,W] f32."""
    full = np.empty((B, NK, C, H, W_IMG), dtype=np.float32)
    for core, o in enumerate(raw_outs):
        # [t, (hh' k), (b c w)] -> [t, hh, k, b, c, w]; hh' = 13-hh
        o = np.asarray(o).reshape(NT, HH, NK, BL, C, W_IMG)[:, ::-1]
        sl = slice(core * BL, (core + 1) * BL)
        v = o.transpose(0, 3, 2, 4, 1, 5)        # [t, b, k, c, hh, w]
        for t in range(NT - 1):
            full[sl, :, :, H0S[t]:H0S[t] + HH, :] = v[t]
        full[sl, :, :, H - (HH - TAIL_HH0):, :] = v[NT - 1][:, :, :,
                                                           TAIL_HH0:, :]
    return full


def get_nc():
    if "nc" not in _CACHE:
        _CACHE["nc"] = _build_nc()
    return _CACHE["nc"]


def kernel(emb, rgb, W, b):
    from concourse.bass_utils import run_bass_kernel_spmd

    emb = np.asarray(emb, dtype=np.float32)
    rgb = np.asarray(rgb, dtype=np.float32)
    W = np.asarray(W, dtype=np.float32)
    b = np.asarray(b, dtype=np.float32)
    assert emb.shape == (B, 128, 8, 8) and rgb.shape == (B, C, H, W_IMG)

    nc = get_nc()
    in_maps = _host_prep(emb, rgb, W, b)
    res = run_bass_kernel_spmd(nc, in_maps, list(range(NCORES)))
    return _assemble([r["out2"] for r in res.results])
